# revision 11
# baseline (speedup 1.0000x reference)
"""Trainium2 Bass kernel for single-head attention with RoPE.

Problem (per full input): x [256, 200, 1024], wq/wk/wv [128, 1024], wo [1024, 128]
  q/k/v = x @ w*.T ; RoPE on q,k (positions 1..S-1, class token 0 unrotated)
  out = softmax(q k^T / sqrt(128)) v @ wo.T

Strategy: data-parallel over batch across 8 NeuronCores (32 batches/core),
software-pipelined over 16 blocks of 2 batches: per iteration emit A(blk)
projections+RoPE, C(blk-2) sums/AV/out-proj/store, B(blk-1) scores/exp/
V-transpose, so the PE always has projection work while ACT/DVE produce
the operands the attention phases need one iteration later. Within B the
scores run BEFORE the V-transposes: the exps they feed gate the next C
phase, while the transposed V has a full iteration of slack.

QKV projections run as 3-term error-compensated fp8 DoubleRow matmuls:
  W*x ~= Wh*xh + Wl*xh + Wh*xl   (the Wl*xl term is ~0.4% and dropped)
with x and 16*W each split into fp8-e4m3 hi+lo pairs host-side (same total
bytes as bf16). DoubleRow contracts K=256 per instruction at 0.5 cyc/row,
so 12 matmuls replace 24 bf16 ones (7200 vs 9600 PE cycles/block). The
16x weight prescale (keeps w out of the fp8 subnormal range) descales for
free: /16 folds into the host cos/sin tables for q,k and into the host
softmax normalization for the v path. Everything else is bf16 (fp8 est/v
for the AV matmul was tested and fails the accuracy gate). Measured rel
err 8.5e-3 vs the 2e-2 gate.

Other structure, per block:
  - x d-major [128, dc*t] hi+lo fp8 via Pool-queue (SWDGE) DMAs, prefetched
    2 blocks ahead (keeps streaming loads off the SP queue, where out-store
    DMAs park holding SEQ while awaiting their evacuations). Projections
    are emitted term-major so the xl transfer overlaps the first 2/3 of
    each accumulation; each weight's hi+lo pair ships as ONE DMA in
    consumption order, with wq's hi half alone and first (it gates the
    very first matmul behind the shared HWDGE/DMA chain).
  - q,k head-major with de-interleaved RoPE; the rotate-half swap runs on
    the DVE as two half-partition muls against a 64-rolled sin table (both
    inputs share a base partition per the hw rule; only the output base is
    shifted). No PE permutation matmul.
  - scores only in [k, q] orientation (feeds AV directly); exp on ACT.
  - softmax row-sums computed transposed on the PE (sums = est^T @ ones,
    moving dim 1); reciprocals ship to the host, which normalizes during
    the unscramble. The sums PSUM tile lives in the out_ps ring: in the
    attn ring its slow release stalled the next block's V-transposes.
  - v seq-major via PE transpose; AV accumulates k-chunks 128+72.
  - out-projection d-major (8 matmuls x 400 moving = 3200 cyc/block);
    evacuations alternate ACT/DVE; output bf16 in SBUF-natural order, one
    DMA per block (quarter stores + DVE av evacs for the final two blocks
    to compress the ACT-bound drain). Host transposes to [B, S, DIM],
    applies rec/16, upcasts to fp32.

Cost-model totals: PE 84.3us busy / ACT 81.8 / DVE 80.5 / DMA 77.5 over a
104.7us span.
"""

import math

import numpy as np
import ml_dtypes

import concourse.bass as bass
import concourse.mybir as mybir
import concourse.tile as tile
from concourse.bass_utils import run_bass_kernel_spmd

B, S, DIM, HD = 256, 200, 1024, 128
BASE = 10000.0
N_CORES = 8
BS = B // N_CORES      # 32 batches per core
BB = 2                 # batches per block
TB = BB * S            # 400 tokens per block
NBLK = BS // BB        # 16 blocks per core
NDC = DIM // 128       # 8 contraction chunks
F32 = mybir.dt.float32
BF16 = mybir.dt.bfloat16
F8 = mybir.dt.float8e4
EXP_SCALE = 1.0 / math.sqrt(HD)
TC = 100               # tokens per output window (4 windows per block)
KC = [(0, 128), (128, S - 128)]   # key chunks within one batch


class _TileContextSplitDrain(tile.TileContext):
    """Workaround: this walrus build rejects >2 sem-wait commands on the
    kernel-tail Drain. Emit each needed wait as its own instruction first."""

    def _drain_and_barrier(self, tick_clock, wait_clock):
        nc = self.nc
        fake = mybir.InstNoOp(
            name=nc.get_next_instruction_name(), ins=[], outs=[],
            engine=mybir.EngineType.SP,
        )
        wait_clock.add_sem_waits(
            fake, tile.ScopedClock({None: tick_clock.global_clock})
        )
        waits = list(fake.sync_info.on_wait) if fake.sync_info is not None else []
        assert self.sems is not None
        handles = {h.name: h for h in self.sems.allocated().values()}
        for w in waits:
            nc.sync.wait_ge(handles[w.ant_name], w.wait_value)
        nc.sync.drain()
        nc.all_engine_barrier()
        popped = nc._tile_sem_poison_stack.pop()
        assert popped is self._sem_poison
        nc.clear_and_free_semaphores(list(self.sems.allocated().values()))
        nc.all_engine_barrier()


def _split_excess_waits(nc):
    """This walrus build accepts 1 sem-wait per instruction (2 on
    EventSemaphore). Tile may attach more; hoist the excess onto standalone
    EventSemaphore instructions right before the owner (same engine, so
    in-order issue preserves the wait semantics)."""
    n = 0
    for b in nc.m.functions[0].blocks:
        insts = b.instructions
        out = []
        for i in insts:
            si = i.sync_info
            if si is not None and len(si.on_wait) > 1:
                keep = 2 if isinstance(i, mybir.InstEventSemaphore) else 1
                waits = list(si.on_wait)
                for w in waits[:-keep] if keep < len(waits) else []:
                    n += 1
                    out.append(mybir.InstEventSemaphore(
                        name=f"{i.name}-evw{n}", ins=[], outs=[],
                        engine=i.engine,
                        sync_info=mybir.SyncInfo(on_wait=[w], on_update=[]),
                    ))
                i.sync_info = mybir.SyncInfo(
                    on_wait=waits[-keep:], on_update=list(si.on_update)
                )
            out.append(i)
        b.instructions = out
    return n


def _build_nc():
    nc = bass.Bass("TRN2", target_bir_lowering=False, debug=False)

    # x and the QKV weights ship as fp8 hi+lo pairs (same total bytes as
    # bf16); projections run as 3-term compensated DoubleRow matmuls:
    # W*x ~= Wh*xh + Wh*xl + Wl*xh with W stored pre-scaled by 16 (the
    # descale folds into the host cos/sin tables and host normalization).
    # hi+lo are PACKED in one tensor so the steady-state load is a single
    # SWDGE generation per block (the gen holds the Pool engine ~1us each).
    xt = nc.dram_tensor("xt", [NBLK, 128, 2 * NDC * TB], F8,
                        kind="ExternalInput").ap()
    w8 = {nm: nc.dram_tensor(nm + "8", [128, 2 * NDC * HD], F8,
                             kind="ExternalInput").ap()
          for nm in ("wq", "wk", "wv")}
    wot = nc.dram_tensor("wot", [HD, DIM], BF16, kind="ExternalInput").ap()
    cosf = nc.dram_tensor("cosf", [128, TB], BF16, kind="ExternalInput").ap()
    sinf = nc.dram_tensor("sinf", [128, TB], BF16, kind="ExternalInput").ap()
    sinfsw = nc.dram_tensor("sinfsw", [128, TB], BF16, kind="ExternalInput").ap()
    ident = nc.dram_tensor("ident", [128, 128], BF16, kind="ExternalInput").ap()
    ones = nc.dram_tensor("ones", [128, 1], BF16, kind="ExternalInput").ap()
    # out is d-major per block: [128 d-partition, dc-chunk, t]; host transposes
    out = nc.dram_tensor("out", [NBLK, 128, NDC * TB], BF16,
                         kind="ExternalOutput").ap()
    # transposed softmax reciprocals [token%100, blk*4+window], accumulated in
    # SBUF and shipped ONCE at kernel end (16 per-block stores cost 16 SP
    # issues + 16 transfers + sem chains right in the drain); host applies
    rec = nc.dram_tensor("rec", [TC, NBLK * 4], F32, kind="ExternalOutput").ap()

    with _TileContextSplitDrain(nc) as tc:
        with (
            tc.tile_pool(name="singles", bufs=1) as singles,
            tc.tile_pool(name="xt", bufs=4) as xt_pool,
            tc.tile_pool(name="qkv_ps", bufs=3, space="PSUM") as qkv_ps,
            tc.tile_pool(name="attn_ps", bufs=2, space="PSUM") as attn_ps,
            tc.tile_pool(name="out_ps", bufs=3, space="PSUM") as out_ps,
            tc.tile_pool(name="ropetmp", bufs=2) as ropetmp,
            tc.tile_pool(name="heads", bufs=3) as heads,
            tc.tile_pool(name="attn_sb", bufs=3) as attn_sb_pool,
            tc.tile_pool(name="outsb", bufs=3) as outsb,
        ):
            # ---- one-time loads (wq-hi + first x first: they gate the first
            # projection matmuls) ----
            # wq's hi half ships alone and first on the SP HWDGE queue,
            # followed immediately by block 0's x chunks (HWDGE generation is
            # a serial global resource; the Pool SWDGE path pays ~1us of
            # descriptor generation per DMA, which delayed the first matmul
            # by ~1.5us). wk/wv/wq-lo ride the ACT queue in consumption order.
            w_sb = {}
            for nm in ("wq", "wk", "wv"):
                t = singles.tile([128, 2 * NDC, HD], F8, name=nm + "8",
                                 tag=nm + "8")
                if nm == "wq":
                    nc.sync.dma_start(out=t[:, 0:NDC, :],
                                      in_=w8[nm][:, 0:NDC * HD])
                    nc.scalar.dma_start(out=t[:, NDC:, :],
                                        in_=w8[nm][:, NDC * HD:])
                else:
                    nc.scalar.dma_start(out=t, in_=w8[nm])
                w_sb[nm] = t
            # block 0's x split per-dc so the first projection matmul starts
            # after ~1 chunk instead of the full 2.3us transfer
            xt0 = xt_pool.tile([128, 2, NDC, TB], F8, name="xt", tag="xt")
            half = NDC // 2
            nc.sync.dma_start(out=xt0[:, 0, 0:half, :],
                              in_=xt[0, :, 0:half * TB])
            nc.sync.dma_start(out=xt0[:, 0, half:, :],
                              in_=xt[0, :, half * TB:NDC * TB])
            nc.sync.dma_start(out=xt0[:, 1, :, :], in_=xt[0, :, NDC * TB:])
            xt_tiles = {0: xt0}

            def ensure_xt(blk):
                if blk in xt_tiles or blk >= NBLK:
                    return
                t = xt_pool.tile([128, 2, NDC, TB], F8, name="xt", tag="xt")
                nc.gpsimd.dma_start(out=t, in_=xt[blk])
                xt_tiles[blk] = t

            ensure_xt(1)
            # rope tables ride the ACT queue behind the weights; ident rides
            # SP behind block 0's x — neither delays the gating transfers
            cos_sb = singles.tile([128, TB], BF16, name="cosf", tag="cosf")
            nc.scalar.dma_start(out=cos_sb, in_=cosf)
            sin_sb = singles.tile([128, TB], BF16, name="sinf", tag="sinf")
            nc.scalar.dma_start(out=sin_sb, in_=sinf)
            # sin table rolled by 64 partitions: the swap-half mul needs both
            # DVE inputs at the SAME base partition (hw verifier rule), with
            # only the output at the shifted base
            sinsw_sb = singles.tile([128, TB], BF16, name="sinfsw", tag="sinfsw")
            nc.scalar.dma_start(out=sinsw_sb, in_=sinfsw)
            id_sb = singles.tile([128, 128], BF16, name="ident", tag="ident")
            nc.sync.dma_start(out=id_sb, in_=ident)
            ones_sb = singles.tile([128, 1], BF16, name="ones", tag="ones")
            wot_sb = singles.tile([HD, DIM], BF16, name="wot", tag="wot")
            # all blocks' softmax reciprocals accumulate here; ONE store at end
            rec_all = singles.tile([TC, NBLK * 4], F32, name="rec", tag="rec")

            # ---- software-pipelined over blocks: A(blk) proj+rope,
            # B(blk-1) transposes/scores/exp, C(blk-2) sums/AV/outproj/store.
            # Keeps the PE fed: while ACT computes exp(scores) for one block,
            # the PE runs the next block's projections instead of stalling.
            def phase_a(blk):
                ensure_xt(blk)
                ensure_xt(blk + 1)
                ensure_xt(blk + 2)
                xt_sb = xt_tiles.pop(blk)
                xh_sb = xt_sb[:, 0]
                xl_sb = xt_sb[:, 1]

                def proj(wname):
                    ps = qkv_ps.tile([128, TB], F32, name="proj_ps", tag="proj_ps")
                    # term-major order: the xl operand is only needed by the
                    # last third of the accumulation, so its transfer overlaps
                    # the first two-thirds
                    wt8 = w_sb[wname]
                    terms = [(0, xh_sb), (NDC, xh_sb), (0, xl_sb)]
                    n = NDC // 2
                    for ti, (b, xt_t) in enumerate(terms):
                        for c in range(n):
                            nc.tensor.matmul(
                                ps,
                                lhsT=wt8[:, b + 2 * c:b + 2 * c + 2, :],
                                rhs=xt_t[:, 2 * c:2 * c + 2, :],
                                start=(c == 0 and ti == 0),
                                stop=(c == n - 1 and ti == 2),
                                perf_mode=mybir.MatmulPerfMode.DoubleRow,
                            )
                    return ps

                q_ps = proj("wq")
                k_ps = proj("wk")
                v_ps = proj("wv")

                # RoPE (de-interleaved rotate-half form). The half-swap
                # swap(q)[p] = q[(p+64)%128] is done by the DVE reading a
                # different base partition than it writes (verified to work
                # on this stack), so no PE permutation matmul is needed.
                # Engine split: the same-partition cos-mul and final add run
                # on the otherwise-idle Pool engine (GPSIMD can't touch PSUM
                # or cross partitions, but these two are SBUF-only and
                # aligned); the partition-crossing sin-muls stay on the DVE.
                def rope(ps, tag):
                    qsb = ropetmp.tile([128, TB], BF16, name="pre_" + tag,
                                       tag="pre_" + tag)
                    nc.scalar.copy(qsb, ps)
                    c = ropetmp.tile([128, TB], BF16, name="rope_c", tag="rope_c")
                    u = ropetmp.tile([128, TB], BF16, name="rope_u", tag="rope_u")
                    h = heads.tile([128, TB], BF16, name=tag, tag=tag)
                    nc.gpsimd.tensor_mul(c, qsb, cos_sb)
                    # sin table is sign-folded ([-sin; +sin]) so one add
                    # completes the rotation
                    nc.vector.tensor_mul(u[0:64, :], qsb[64:128, :],
                                         sinsw_sb[64:128, :])
                    nc.vector.tensor_mul(u[64:128, :], qsb[0:64, :],
                                         sinsw_sb[0:64, :])
                    nc.gpsimd.tensor_add(h, c, u)
                    return h

                q_h = rope(q_ps, "q_h")
                k_h = rope(k_ps, "k_h")
                v_h = heads.tile([128, TB], BF16, name="v_h", tag="v_h")
                nc.scalar.copy(v_h, v_ps)
                return dict(q_h=q_h, k_h=k_h, v_h=v_h)

            def phase_b(st):
                q_h, k_h, v_h = st["q_h"], st["k_h"], st["v_h"]
                # scores [k, q] + exp
                est_sb = []
                for kc, (k0, ksz) in enumerate(KC):
                    stp = attn_ps.tile([128, TB], F32, name=f"st{kc}_ps",
                                       tag="attn_ps")
                    for i in range(BB):
                        nc.tensor.matmul(
                            stp[0:ksz, i * S:(i + 1) * S],
                            lhsT=k_h[:, i * S + k0: i * S + k0 + ksz],
                            rhs=q_h[:, i * S:(i + 1) * S],
                            start=True, stop=True,
                        )
                    e = attn_sb_pool.tile([128, TB], BF16, name=f"est{kc}",
                                          tag=f"est{kc}")
                    nc.scalar.activation(
                        out=e[0:ksz, :], in_=stp[0:ksz, :],
                        func=mybir.ActivationFunctionType.Exp,
                        scale=EXP_SCALE,
                    )
                    est_sb.append(e)
                # V -> seq-major [k, h] via PE transpose
                vt_ps = [
                    attn_ps.tile([128, BB * 128], BF16, name=f"vt{kc}_ps",
                                 tag="attn_ps")
                    for kc in range(2)
                ]
                for i in range(BB):
                    for kc, (k0, ksz) in enumerate(KC):
                        nc.tensor.transpose(
                            vt_ps[kc][0:ksz, i * 128:(i + 1) * 128],
                            v_h[:, i * S + k0: i * S + k0 + ksz], id_sb,
                        )
                vt_sb = []
                for kc, (k0, ksz) in enumerate(KC):
                    t = attn_sb_pool.tile([128, BB * 128], BF16,
                                          name=f"vt{kc}_sb", tag=f"vt{kc}_sb")
                    nc.vector.tensor_copy(t[0:ksz, :], vt_ps[kc][0:ksz, :])
                    vt_sb.append(t)

                st["vt_sb"] = vt_sb
                st["est_sb"] = est_sb

            def phase_c(st, blk, last):
                vt_sb, est_sb = st["vt_sb"], st["est_sb"]
                # transposed row-sums on the PE: sums[q,1] = est^T @ ones
                sums_ps = out_ps.tile([TC, 4], F32, name="sums_ps",
                                      tag="out_ps")
                for w in range(4):
                    i, tcw = divmod(w, 2)
                    t0 = i * S + tcw * TC
                    for kc, (k0, ksz) in enumerate(KC):
                        nc.tensor.matmul(
                            sums_ps[0:TC, w:w + 1],
                            lhsT=est_sb[kc][0:ksz, t0:t0 + TC],
                            rhs=ones_sb[0:ksz, :],
                            start=(kc == 0), stop=(kc == 1),
                        )
                nc.vector.reciprocal(rec_all[:, 4 * blk:4 * blk + 4], sums_ps)

                # AV: attn_head[h, q] (unnormalized)
                av_ps = attn_ps.tile([128, TB], F32, name="av_ps",
                                     tag="attn_ps")
                for i in range(BB):
                    for kc, (k0, ksz) in enumerate(KC):
                        nc.tensor.matmul(
                            av_ps[:, i * S:(i + 1) * S],
                            lhsT=vt_sb[kc][0:ksz, i * 128:(i + 1) * 128],
                            rhs=est_sb[kc][0:ksz, i * S:(i + 1) * S],
                            start=(kc == 0), stop=(kc == 1),
                        )
                av_sb = attn_sb_pool.tile([128, TB], BF16, name="av_sb",
                                          tag="av_sb")
                if blk >= NBLK - 2:
                    # drain is ACT-bound: balance by moving av to the DVE
                    nc.vector.tensor_copy(av_sb, av_ps)
                else:
                    nc.scalar.copy(av_sb, av_ps)

                # output projection, d-major: out[dc*128+p, t] = wo @ av
                # (3200 PE cycles/block vs 4096 token-major); softmax
                # normalization moves to the host via the shipped rec tile.
                osb = outsb.tile([128, NDC * TB], BF16, name="osb", tag="osb")
                for dc in range(NDC):
                    ops = out_ps.tile([128, TB], F32, name="out_ps",
                                      tag="out_ps")
                    nc.tensor.matmul(
                        ops,
                        lhsT=wot_sb[:, dc * 128:(dc + 1) * 128],
                        rhs=av_sb,
                        start=True, stop=True,
                    )
                    dst = osb[:, dc * TB:(dc + 1) * TB]
                    # 3 ACT / 5 DVE: with the rope cos-mul+add moved to Pool,
                    # the DVE has headroom and the ACT was the fuller engine
                    if dc in (0, 3, 6):
                        nc.scalar.copy(dst, ops)
                    else:
                        nc.vector.tensor_copy(dst, ops)
                    if blk >= NBLK - 2 and dc % 2 == 1:
                        # tail latency: ship each quarter as soon as it's
                        # done (both final blocks, so the device interleaves
                        # them instead of one 2.3us transfer blocking)
                        h0 = (dc - 1) * TB
                        nc.sync.dma_start(
                            out=out[blk, :, h0:h0 + 2 * TB],
                            in_=osb[:, h0:h0 + 2 * TB],
                        )
                if blk < NBLK - 2:
                    nc.sync.dma_start(out=out[blk], in_=osb)

            states = {}
            for blk in range(NBLK + 2):
                if blk == 2:
                    nc.sync.dma_start(out=ones_sb, in_=ones)
                    nc.sync.dma_start(out=wot_sb, in_=wot)
                if blk < NBLK:
                    states[blk] = phase_a(blk)
                if blk < NBLK:
                    if blk - 2 >= 0:
                        phase_c(states[blk - 2], blk - 2, last=False)
                        del states[blk - 2]
                    if blk - 1 >= 0:
                        phase_b(states[blk - 1])
                else:
                    # drain: C first — it gates the output stores, while B
                    # only feeds the final C one iteration later
                    phase_c(states[blk - 2], blk - 2, last=(blk - 2 == NBLK - 1))
                    del states[blk - 2]
                    if blk - 1 < NBLK:
                        phase_b(states[blk - 1])
            nc.sync.dma_start(out=rec, in_=rec_all)
    _split_excess_waits(nc)
    return nc


_NC_CACHE = {}


def _get_nc():
    if "nc" not in _NC_CACHE:
        _NC_CACHE["nc"] = _build_nc()
    return _NC_CACHE["nc"]


def _host_prep(x, wq, wk, wv, wo):
    """Shared (non-x) device inputs + per-core x^T shards."""
    bf = ml_dtypes.bfloat16
    f8 = ml_dtypes.float8_e4m3fn
    perm = np.concatenate([np.arange(0, HD, 2), np.arange(1, HD, 2)])
    # weight layout [p, dc*h]: row d of w.T at (p=d%128, dc=d//128);
    # stored as fp8 hi/lo pair of 16*w (descale folds into cos/sin + host
    # normalization)
    def wlayout(w):
        return np.ascontiguousarray(
            w.T.reshape(NDC, 128, HD).transpose(1, 0, 2).reshape(128, NDC * HD)
        )
    def split8(a):
        h = a.astype(f8)
        l = (a - h.astype(np.float32)).astype(f8)
        return h, l
    w8 = {}
    for nm, w in (("wq", wq[perm]), ("wk", wk[perm]), ("wv", wv)):
        h, l = split8(16.0 * wlayout(w))
        w8[nm + "8"] = np.ascontiguousarray(np.concatenate([h, l], axis=1))
    wot = np.ascontiguousarray(wo.T).astype(bf)

    inv_freq = 1.0 / BASE ** (np.arange(0, HD, 2, dtype=np.float64) / HD)
    ang = np.zeros((S, HD // 2), np.float64)
    ang[1:] = np.arange(S - 1, dtype=np.float64)[:, None] * inv_freq[None, :]
    # /16 compensates the 16x-scaled fp8 projection weights for q,k
    cos_t = (np.cos(ang).T / 16.0).astype(np.float32)   # [64, S]
    sin_t = (np.sin(ang).T / 16.0).astype(np.float32)
    cosf = np.tile(np.concatenate([cos_t, cos_t], axis=0), (1, BB))  # [128, TB]
    # sign-folded: rotated = q*cosf + swap64(q)*sinf in one add
    sinf = np.tile(np.concatenate([-sin_t, sin_t], axis=0), (1, BB))

    shared = dict(w8)
    shared.update({
        "wot": wot,
        "cosf": np.ascontiguousarray(cosf).astype(bf),
        "sinf": np.ascontiguousarray(sinf).astype(bf),
        "sinfsw": np.ascontiguousarray(np.roll(sinf, 64, axis=0)).astype(bf),
        "ident": np.eye(128, dtype=np.float32).astype(bf),
        "ones": np.ones((128, 1), np.float32).astype(bf),
    })
    x8h, x8l = split8(x)
    xts = []
    for c in range(N_CORES):
        def lay(a):
            xc = a[c * BS:(c + 1) * BS].reshape(NBLK, TB, NDC, 128)
            return xc.transpose(0, 3, 2, 1).reshape(NBLK, 128, NDC * TB)
        # hi+lo packed per block: one SWDGE load per block on-device
        xts.append(np.ascontiguousarray(
            np.concatenate([lay(x8h), lay(x8l)], axis=2)))
    return shared, xts


def kernel(x, wq, wk, wv, wo):
    x = np.asarray(x, np.float32)
    wq = np.asarray(wq, np.float32)
    wk = np.asarray(wk, np.float32)
    wv = np.asarray(wv, np.float32)
    wo = np.asarray(wo, np.float32)

    shared, xts = _host_prep(x, wq, wk, wv, wo)
    in_maps = [dict(shared, xt=xts[c]) for c in range(N_CORES)]
    nc = _get_nc()
    res = run_bass_kernel_spmd(nc, in_maps, list(range(N_CORES)))
    outs = []
    for c in range(N_CORES):
        # out: [NBLK, 128 d-part, NDC, TB] bf16 (unnormalized);
        # rec: [TC, NBLK*4] f32, window w=(batch i, chunk tcw), token
        #      w*TC + p within the block
        o = np.asarray(res.results[c]["out"]).astype(np.float32)
        o = o.reshape(NBLK, 128, NDC, TB).transpose(0, 3, 2, 1)  # [blk,t,dc,p]
        o = o.reshape(NBLK, TB, DIM)
        r = np.asarray(res.results[c]["rec"])
        r = r.reshape(TC, NBLK, 4).transpose(1, 2, 0)            # [blk,w,p]
        o *= r.reshape(NBLK, TB, 1) / 16.0
        outs.append(o.reshape(BS, S, DIM))
    return np.concatenate(outs, axis=0)

